# revision 3
# baseline (speedup 1.0000x reference)
"""Trainium2 Bass kernel for the batch ConsistencyLoss (masked pairwise KL).

Math (reference):
    emb = x / ||x||;  sim = emb @ emb.T;  mask = (sim > 0.8) & ~eye
    L = log_softmax(routing);  P = exp(L);  ne[j] = sum_k P[j,k] L[j,k]
    kl[i,j] = ne[j] - (L @ P.T)[i,j]
    loss = sum(mask * kl) / count(mask)

v2 device algorithm (per core, SPMD over 8 cores):
  * Triangle-symmetric decomposition over 16 column strips of 512 rows.
    Core d owns strips d ("half A", embt cols 0:512 after the host
    permutation) and d+8 ("half B", cols 512:1024).  Each unordered
    strip pair is computed ONCE globally; both KL directions of an
    off-diagonal pair are accumulated in the same pass through a
    36-column stationary table W36 = [-P |ne | 1 | L | 1] (the L/1 tail
    zeroed for diagonal pairs, which already cover both orders).
  * Rows are normalized to ||u|| = 16 and quantized to fp8e4 BEFORE the
    transpose; 4 fp8 values are packed per fp32 element so each 128-row
    tile needs only TWO PE transposes (fp32, bit-exact pass-through)
    instead of eight.  The resulting layout embt[p, q, 4b+r] (h =
    512q + 4p + r) feeds DoubleRow fp8 matmuls via stride-4 APs.
  * Masks are +-0.5-encoded ((sim>204.8) - 0.5) so they can be produced
    on EITHER engine: DVE via tensor_scalar(is_gt, subtract), ACT via
    Sign (+-1) against a HALVED stationary table W36h.  One V [36,1024]
    PSUM accumulator; the -0.5 offset is corrected in the readout with
    T[k] = sum_blocks sum_j Weff[j,k], accumulated by N=1 ones-matmuls
    that reuse each block's already-loaded stationary.
  * count = sum_i V[17] + V[35] (dir-2 ones column is zeroed for
    diagonal pairs so off-diagonal entries count twice = both orders);
    host subtracts B for the diagonal pairs (kl there is 0 exactly).
"""

import numpy as np

import concourse.bacc as bacc
import concourse.tile as tile
from concourse import mybir
from concourse.bass_utils import run_bass_kernel_spmd
from concourse.masks import make_identity

B, E, H = 8192, 16, 1024
NCORES = 8
STRIP = 1024         # rows per core (2 half-strips of 512)
MT = STRIP // 128    # 8 row chunks per core strip
BT = B // 128        # 64 batch tiles
NPOS = 16            # 512-row strip positions
WW = 36              # stationary width [-P(16) | ne | 1 | L(16) | 1]
SIM_THRESHOLD = 0.8
SCALE = 16.0
THRESH = SIM_THRESHOLD * SCALE * SCALE  # 204.8
WEIGHT = 1.0
F32 = mybir.dt.float32
BF16 = mybir.dt.bfloat16
F8 = mybir.dt.float8e4
AX = mybir.AxisListType.X
OP = mybir.AluOpType
AF = mybir.ActivationFunctionType
DR = mybir.MatmulPerfMode.DoubleRow

# static block schedule: (ready_step, ihalf, t, bt) -- bt is the global
# 128-row block index of the j-strip sub-block.  Pair sets (position
# space, identical on every core thanks to the host permutation):
#   half A (cols 0:512)    x strips t = 0..8   (t=0 is the diagonal)
#   half B (cols 512:1024) x strips t = 1,9..15 (t=1 is the diagonal)
def _block_schedule():
    sched = []
    for t in range(NPOS):
        ready = 4 * t + 4
        if t <= 8:
            for j in range(4):
                sched.append((max(ready, 4), 0, t, 4 * t + j))
        if t == 1 or t >= 9:
            for j in range(4):
                sched.append((max(ready, 8), 1, t, 4 * t + j))
    sched.sort(key=lambda b: b[0])
    return sched


SCHED = _block_schedule()
N_A = sum(1 for b in SCHED if b[1] == 0)   # 36
N_B = sum(1 for b in SCHED if b[1] == 1)   # 32


def _kernel(tc, emb, rp, out_dram, reps=1, loop_iters=None, phases="ABCD",
            dbg=None):
    nc = tc.nc
    with tc.tile_pool(name="persist", bufs=1) as persist:
        # packed q(u)^T: [p, strip, q, 4b_local+r] fp8, h = 512q + 4p + r
        # (strip-chunked so the DoubleRow Ko stride fits the 16-bit ISA
        # step field: q-pair stride = 2048 B)
        embt = persist.tile([128, NPOS, 2, 4 * 512], F8)
        W36f = persist.tile([128, BT, WW], BF16)
        W36h = persist.tile([128, BT, WW], BF16)   # 0.5x, for ACT masks
        W36d = persist.tile([128, 8, WW], BF16)    # diag: L/1 tail zeroed
        LTfull = persist.tile([WW, STRIP], F32)
        identf = persist.tile([128, 128], F32)
        ones36 = persist.tile([WW, 1], F32)
        onesb = persist.tile([128, 1], BF16)
        halfb = persist.tile([128, 1], BF16)
        nthr = persist.tile([128, 1], F32)
        make_identity(nc, identf)
        nc.vector.memset(ones36, 1.0)
        nc.vector.memset(onesb, 1.0)
        nc.vector.memset(halfb, 0.5)
        nc.vector.memset(nthr, -THRESH)
        if "B" not in phases and "C" in phases:
            nc.gpsimd.memset(embt, 0.02)
        if "A" not in phases and "C" in phases:
            nc.gpsimd.memset(W36f, 0.01)
            nc.gpsimd.memset(W36h, 0.005)
            nc.gpsimd.memset(W36d, 0.01)
            nc.gpsimd.memset(LTfull, 0.01)

        args = (tc, nc, emb, rp, out_dram, embt, W36f, W36h, W36d,
                LTfull, identf, ones36, onesb, halfb, nthr, dbg)
        if loop_iters is not None:
            with tc.For_i(0, loop_iters, 1):
                _phases(*args, "", phases)
            return
        for rep in range(reps):
            _phases(*args, f"r{rep}_" if reps > 1 else "", phases)


def _phases(tc, nc, emb, rp, out_dram, embt, W36f, W36h, W36d,
            LTfull, identf, ones36, onesb, halfb, nthr, dbg, r,
            which="ABCD"):
    # ---- Phase A: softmax stats + stationary/readout tables ----
    # The [128, BT]-wide head runs up front; the per-bt table writes are
    # emitted by a_tail_step(bt) from inside the BC loop so the DVE work
    # overlaps the prep/sim pipeline (each table lands one step before
    # its first stationary use at s = 4t+4).
    do_a = "A" in which
    smx_pool = tc.tile_pool(name=f"{r}smx", bufs=1)
    smx = smx_pool.__enter__()
    st_a = {}
    if do_a:
        rp_sb = smx.tile([128, BT, E], F32, tag="rp_sb")
        nc.gpsimd.dma_start(
            out=rp_sb, in_=rp.rearrange("(bt p) e -> p bt e", p=128))
        # Logits are N(0,1): skip the max-shift, batch all Exp under one
        # ACT table.  L = x - logs;  ne = (sum e*x)/s - logs.
        e_all = smx.tile([128, BT, E], F32, tag="e_all")
        s_all = smx.tile([128, BT], F32, tag="s_all")
        logs_all = smx.tile([128, BT], F32, tag="logs_all")
        rs_all = smx.tile([128, BT], F32, tag="rs_all")
        nc.scalar.activation(out=e_all, in_=rp_sb, func=AF.Exp)
        nc.vector.reduce_sum(out=s_all, in_=e_all, axis=AX)
        nc.scalar.activation(out=logs_all, in_=s_all, func=AF.Ln)
        nc.vector.reciprocal(out=rs_all, in_=s_all)
        prodel = smx.tile([128, BT, E], F32, tag="prodel")
        nc.vector.tensor_tensor(out=prodel, in0=e_all, in1=rp_sb, op=OP.mult)
        epx = smx.tile([128, BT], F32, tag="epx")
        nc.vector.reduce_sum(out=epx, in_=prodel, axis=AX)
        ne_all = smx.tile([128, BT], F32, tag="ne_all")
        nc.vector.tensor_tensor(out=ne_all, in0=epx, in1=rs_all, op=OP.mult)
        nc.vector.tensor_tensor(out=ne_all, in0=ne_all, in1=logs_all,
                                op=OP.subtract)
        # W36f cols: 0:16 = -P, 16 = ne, 17 = 1, 18:34 = L, 34 = pad,
        # 35 = 1 (col 34 must be zeroed -- it pairs with LTfull row 34)
        nc.vector.memset(W36f[:, :, 17], 1.0)
        nc.vector.memset(W36f[:, :, 34], 0.0)
        nc.vector.memset(W36f[:, :, 35], 1.0)
        with nc.allow_low_precision(reason="bf16 stationary tables"):
            nc.vector.tensor_copy(
                out=W36f[:, :, 16:17],
                in_=ne_all.rearrange("p (t o) -> p t o", o=1))
        st_a.update(rp_sb=rp_sb, e_all=e_all, logs_all=logs_all,
                    rs_all=rs_all, ne_all=ne_all)

    def a_tail_step(bt):
        if not do_a:
            return
        rp_sb, e_all = st_a["rp_sb"], st_a["e_all"]
        logs_all, rs_all = st_a["logs_all"], st_a["rs_all"]
        ne_all = st_a["ne_all"]
        with nc.allow_low_precision(reason="bf16 stationary tables"):
            nc.vector.tensor_scalar(W36f[:, bt, 0:16], e_all[:, bt, :],
                                    rs_all[:, bt:bt + 1], -1.0,
                                    op0=OP.mult, op1=OP.mult)
            nc.vector.tensor_scalar(W36f[:, bt, 18:34], rp_sb[:, bt, :],
                                    logs_all[:, bt:bt + 1], None,
                                    op0=OP.subtract)
            nc.vector.tensor_scalar(W36h[:, bt, :], W36f[:, bt, :], 0.5,
                                    None, op0=OP.mult)
            if bt < 8:
                nc.vector.tensor_copy(out=W36d[:, bt, :],
                                      in_=W36f[:, bt, :])
                nc.vector.memset(W36d[:, bt, 18:36], 0.0)
    def build_lt(ltpool):
        # LTfull rows: 0:16 = L^T, 16 = 1, 17 = 0, 18:34 = -P^T, 35 = ne^T
        if not do_a:
            return
        rp_sb, e_all = st_a["rp_sb"], st_a["e_all"]
        logs_all, rs_all = st_a["logs_all"], st_a["rs_all"]
        ne_all = st_a["ne_all"]
        for ms in range(MT):
            Lm = smx.tile([128, WW], F32, tag="Lm", bufs=3)
            nc.vector.memset(Lm[:, 16:17], 1.0)
            nc.vector.memset(Lm[:, 17:18], 0.0)
            nc.vector.memset(Lm[:, 34:35], 0.0)
            nc.vector.tensor_scalar(Lm[:, 0:16], rp_sb[:, ms, :],
                                    logs_all[:, ms:ms + 1], None,
                                    op0=OP.subtract)
            nc.vector.tensor_scalar(Lm[:, 18:34], e_all[:, ms, :],
                                    rs_all[:, ms:ms + 1], -1.0,
                                    op0=OP.mult, op1=OP.mult)
            nc.vector.tensor_copy(out=Lm[:, 35:36], in_=ne_all[:, ms:ms + 1])
            lt = ltpool.tile([WW, 128], F32, tag="lt", bufs=2)
            nc.tensor.matmul(out=lt, lhsT=Lm, rhs=identf,
                             start=True, stop=True)
            nc.scalar.copy(out=LTfull[:, ms * 128:(ms + 1) * 128], in_=lt)

    if do_a and "C" not in which and "B" not in which:
        with tc.tile_pool(name=f"{r}ltp", bufs=2, space="PSUM") as ltps:
            for bt in range(BT):
                a_tail_step(bt)
            build_lt(ltps)

    # ---- Phase B+C: merged prep + triangle sim pipeline ----
    do_b = "B" in which
    do_c = "C" in which
    if not (do_b or do_c):
        smx_pool.__exit__(None, None, None)
        return
    embtf = embt.bitcast(F32)  # [128, 16, 2, 512] fp32 view (4 fp8/elem)
    with tc.tile_pool(name=f"{r}vps", bufs=1, space="PSUM") as vps, \
         tc.tile_pool(name=f"{r}tps", bufs=1, space="PSUM") as tps:
      V = vps.tile([WW, STRIP], F32, name="V") if do_c else None
      T = tps.tile([WW, STRIP], F32, name="T") if do_c else None
      with tc.tile_pool(name=f"{r}embp", bufs=3) as ep, \
           tc.tile_pool(name=f"{r}trps", bufs=2, space="PSUM") as trps, \
           tc.tile_pool(name=f"{r}simps", bufs=2, space="PSUM") as sps, \
           tc.tile_pool(name=f"{r}mkp", bufs=8) as mkp:
        tpend = []   # pending transpose copy-outs (tp, bt)
        vpend = []   # pending V/T matmuls (ihalf, wsrc, msk)
        state = dict(na=0, nb=0, nt=0, nmask=0)

        def drain_tp():
            tp_, bt_ = tpend.pop(0)
            eng = nc.scalar.copy if bt_ % 2 == 0 else nc.vector.tensor_copy
            st, jl = bt_ // 4, bt_ % 4
            eng(out=embtf[:, st, :, jl * 128:(jl + 1) * 128],
                in_=tp_.rearrange("p (q c) -> p q c", q=2))

        def prep(bt):
            x = ep.tile([128, H], F32, tag="ex", bufs=6)
            dma_eng = nc.sync if bt % 2 == 0 else nc.gpsimd
            dma_eng.dma_start(out=x, in_=emb[bt * 128:(bt + 1) * 128, :])
            scr = ep.tile([128, H], BF16, tag="sqscr", bufs=2)
            ss = ep.tile([128, 1], F32, tag="ss", bufs=3)
            nc.scalar.activation(out=scr, in_=x, func=AF.Square,
                                 accum_out=ss)
            n16 = ep.tile([128, 1], F32, tag="n16", bufs=3)
            nc.scalar.activation(out=n16, in_=ss, func=AF.Sqrt, bias=0.0,
                                 scale=1.0 / (SCALE * SCALE))
            rs = ep.tile([128, 1], F32, tag="rs", bufs=3)
            nc.vector.reciprocal(out=rs, in_=n16)
            xq = ep.tile([128, H], F8, tag="xq", bufs=3)
            with nc.allow_low_precision(reason="fp8 quantized embeddings"):
                nc.vector.tensor_scalar(xq, x, rs, None, op0=OP.mult)
            if tpend:
                drain_tp()
            xqf = xq.bitcast(F32)  # [128, 256]
            tp = trps.tile([128, 256], F32, tag="tr")
            for q in range(2):
                nc.tensor.transpose(tp[:, q * 128:(q + 1) * 128],
                                    xqf[:, q * 128:(q + 1) * 128], identf)
            tpend.append((tp, bt))

        def drain_v(stop_all=False):
            ihalf, wsrc, msk, tvec = vpend.pop(0)
            if ihalf == 0:
                state["na"] += 1
                start, stop = state["na"] == 1, state["na"] == N_A
            else:
                state["nb"] += 1
                start, stop = state["nb"] == 1, state["nb"] == N_B
            nc.tensor.matmul(out=V[:, ihalf * 512:(ihalf + 1) * 512],
                             lhsT=wsrc, rhs=msk, start=start, stop=stop)
            tcol = ihalf * 512
            nc.tensor.matmul(out=T[:, tcol:tcol + 1], lhsT=wsrc, rhs=tvec,
                             start=start, stop=stop)

        def sim_mms(blk):
            ihalf, t, bt = blk[1], blk[2], blk[3]
            simT = sps.tile([128, 512], F32, tag="simT")
            jl = bt % 4
            for g, r4 in enumerate(range(4)):
                j0 = 4 * (jl * 128) + r4
                lhsT = embt[:, t, :, j0:j0 + 4 * 127 + 1:4]
                rhs = embt[:, ihalf, :, r4:r4 + 4 * 511 + 1:4]
                nc.tensor.matmul(out=simT, lhsT=lhsT, rhs=rhs,
                                 start=(g == 0), stop=(g == 3),
                                 perf_mode=DR)
            if vpend:
                drain_v()
            return simT

        def mask_and_queue(blk, simT):
            ihalf, t, bt = blk[1], blk[2], blk[3]
            is_diag = (ihalf == 0 and t == 0) or (ihalf == 1 and t == 1)
            msk = mkp.tile([128, 512], BF16, tag="mask")
            if is_diag:
                use_dve = True
                wsrc = W36d[:, bt, :]
            else:
                state["nmask"] += 1
                use_dve = state["nmask"] % 3 == 0
                wsrc = (W36f if use_dve else W36h)[:, bt, :]
            with nc.allow_low_precision(reason="bf16 masks"):
                if use_dve:
                    nc.vector.tensor_scalar(msk, simT, THRESH, 0.5,
                                            op0=OP.is_gt, op1=OP.subtract)
                else:
                    nc.scalar.activation(out=msk, in_=simT, func=AF.Sign,
                                         bias=nthr)
            vpend.append((ihalf, wsrc, msk, halfb if use_dve else onesb))

        ptr = 0
        for s in range(BT):
            due = []
            while (do_c and ptr < len(SCHED) and SCHED[ptr][0] <= s
                   and len(due) < 2):
                due.append(SCHED[ptr])
                ptr += 1
            simTs = [sim_mms(b) for b in due]
            if do_b:
                prep(s)
            a_tail_step(s)
            for b, st in zip(due, simTs):
                mask_and_queue(b, st)
        while do_c and ptr < len(SCHED):
            blk = SCHED[ptr]
            ptr += 1
            st = sim_mms(blk)
            mask_and_queue(blk, st)
        while tpend:
            drain_tp()
        if do_c:
            while vpend:
                drain_v()

      # ---- readout ----
      if do_c:
        with tc.tile_pool(name=f"{r}fin", bufs=1) as fin, \
             tc.tile_pool(name=f"{r}fps", bufs=1, space="PSUM") as fps:
            build_lt(fps)
            Vs = fin.tile([WW, STRIP], F32)
            nc.scalar.copy(out=Vs, in_=V)
            Ts = fin.tile([WW, 2], F32)
            nc.vector.tensor_copy(out=Ts[:, 0:1], in_=T[:, 0:1])
            nc.vector.tensor_copy(out=Ts[:, 1:2], in_=T[:, 512:513])
            Vc = fin.tile([WW, STRIP], F32)
            nc.vector.tensor_scalar(Vc[:, 0:512], Vs[:, 0:512],
                                    Ts[:, 0:1], None, op0=OP.add)
            nc.vector.tensor_scalar(Vc[:, 512:1024], Vs[:, 512:1024],
                                    Ts[:, 1:2], None, op0=OP.add)
            scr = fin.tile([WW, STRIP], F32)
            nc.vector.tensor_tensor(out=scr, in0=Vc, in1=LTfull, op=OP.mult)
            accs = fin.tile([WW, 2], F32)
            nc.vector.reduce_sum(out=accs[:, 0:1], in_=scr, axis=AX)
            nc.vector.reduce_sum(out=accs[:, 1:2], in_=Vc, axis=AX)
            # sel col0 = 1 (masked sum over all rows); col1 = e17 + e35
            sel = fin.tile([WW, 2], F32)
            nc.vector.tensor_copy(out=sel[:, 0:1], in_=ones36)
            nc.vector.tensor_tensor(out=sel[:, 1:2],
                                    in0=identf[0:WW, 17:18],
                                    in1=identf[0:WW, 35:36], op=OP.add)
            msel = fin.tile([WW, 2], F32)
            nc.vector.tensor_tensor(out=msel, in0=accs, in1=sel, op=OP.mult)
            res = fps.tile([1, 2], F32)
            nc.tensor.matmul(out=res, lhsT=ones36, rhs=msel,
                             start=True, stop=True)
            out_sb = fin.tile([1, 2], F32)
            nc.scalar.copy(out=out_sb, in_=res)
            nc.sync.dma_start(out=out_dram, in_=out_sb)
            if dbg is not None:
                nc.sync.dma_start(out=dbg["embt"], in_=embt)
                nc.sync.dma_start(out=dbg["W36f"], in_=W36f)
                nc.sync.dma_start(out=dbg["LTfull"], in_=LTfull)
                nc.sync.dma_start(out=dbg["Vs"], in_=Vs)
                nc.sync.dma_start(out=dbg["Ts"], in_=Ts)
    smx_pool.__exit__(None, None, None)


def build_bass(reps=1, loop_iters=None, phases="ABCD", debug_dump=False):
    nc = bacc.Bacc("TRN2", target_bir_lowering=False, debug=False)
    emb = nc.dram_tensor("emb", [B, H], F32, kind="ExternalInput").ap()
    rp = nc.dram_tensor("rp", [B, E], F32, kind="ExternalInput").ap()
    out = nc.dram_tensor("out", [1, 2], F32, kind="ExternalOutput").ap()
    dbg = None
    if debug_dump:
        dbg = {
            "embt": nc.dram_tensor("d_embt", [128, NPOS, 2, 4 * 512], F8,
                                   kind="ExternalOutput").ap(),
            "W36f": nc.dram_tensor("d_w36f", [128, BT, WW], BF16,
                                   kind="ExternalOutput").ap(),
            "LTfull": nc.dram_tensor("d_lt", [WW, STRIP], F32,
                                     kind="ExternalOutput").ap(),
            "Vs": nc.dram_tensor("d_vs", [WW, STRIP], F32,
                                 kind="ExternalOutput").ap(),
            "Ts": nc.dram_tensor("d_ts", [WW, 2], F32,
                                 kind="ExternalOutput").ap(),
        }
    with tile.TileContext(nc) as tc:
        _kernel(tc, emb, rp, out, reps=reps,
                loop_iters=loop_iters, phases=phases, dbg=dbg)
    nc.compile()
    return nc


_NC_CACHE = None


def make_in_maps(rp, emb):
    """Per-core inputs: rows reordered to [strip d, strip d+8,
    strips d+1..d+7, strips d+9..d+15] (strips of 512, cyclic mod 16)."""
    in_maps = []
    for d in range(NCORES):
        s_list = ([d, (d + 8) % 16]
                  + [(d + k) % 16 for k in range(1, 8)]
                  + [(d + k) % 16 for k in range(9, 16)])
        order = np.concatenate(
            [np.arange(512 * s, 512 * (s + 1)) for s in s_list])
        in_maps.append({
            "emb": np.ascontiguousarray(emb[order]),
            "rp": np.ascontiguousarray(rp[order]),
        })
    return in_maps


def kernel(routing_probs: np.ndarray, input_embeddings: np.ndarray,
           **_unused) -> np.ndarray:
    global _NC_CACHE
    if _NC_CACHE is None:
        _NC_CACHE = build_bass()
    nc = _NC_CACHE
    rp = np.ascontiguousarray(routing_probs, dtype=np.float32)
    emb = np.ascontiguousarray(input_embeddings, dtype=np.float32)
    in_maps = make_in_maps(rp, emb)
    res = run_bass_kernel_spmd(nc, in_maps, core_ids=list(range(NCORES)))
    vals = np.array([r["out"].reshape(2) for r in res.results],
                    dtype=np.float64)
    total = vals[:, 0].sum()
    cnt = vals[:, 1].sum() - B  # drop the diagonal pairs (kl there is 0)
    if cnt > 0:
        loss = np.float32(total) / np.float32(max(cnt, 1.0))
    else:
        loss = 0.0
    return np.array(WEIGHT * loss, dtype=np.float32)
